# revision 28
# baseline (speedup 1.0000x reference)
"""CrossAttentionFusion Bass kernel v2 — batch-pair / token-major hybrid.

Reference (T=4096, B=64, D=64):
    q = eeg @ Wq.T + bq ; k = fnirs @ Wk.T + bk ; v = fnirs @ Wv.T + bv
    s = sum(q*k, -1) * D**-0.5 ; a = softmax(s, axis=0) ; out = eeg + a*v

Algebra: s = x^T G y + w.x + u.y, G = SCALE*Wq^T Wk, w = SCALE*Wq^T bk,
u = SCALE*Wk^T bq.

Core ideas (per core: 8 batches, all 4096 tokens):
  - Batches processed in PAIRS (b0,b1) stacked on partitions: host packs
    XP=[x0;x1], YP=[y0;y1] (feature-major bf16 [128, tok]).
  - PE: z-pair = blockdiag(G,G)^T @ XP   [128=z0|z1, 1024] one N=1024 mm.
  - DVE: m = (z + u2) * YP  — ONE scalar_tensor_tensor per 1024 tokens
    (PSUM x SBUF, u per-partition in this layout), bf16 out.
  - PE reduces scores: per 128-token slice, lhsT=m-tile with rhs
    [1_64;0] / [0;1_64] (N=1) accumulated with w.x via rhs [w;0] / [0;w]
    second matmul into a per-token scores PSUM bank [128tok, 256 cols].
  - v-pair = YP-slice^T @ [[Wv^T],[0]] / [[0],[Wv^T]] (token-major v),
    + bv via ones-row matmul; scalar ACT evacuates to bf16 vstore.
  - Softmax over T per batch: no max-sub (|s|~4); exp ACT reads scores
    PSUM with accum_out; GpSimd C-reduce -> Z_b; reciprocal;
    gpsimd.partition_broadcast for 1/Z.
  - Pass B token-major: ABT_b = E_b*(1/Z_b) broadcast across 64 feats
    (ONE stride-0 scalar ACT per batch), av = ABT*v (DVE), o = av + x
    (XT host-packed token-major bf16; DVE/GpSimd alternating), DMA out
    on the scalar HWDGE ring (inputs ride the sync ring).
  - Two batch-half pipeline: softmax+passB of pairs {0,1} overlaps
    pass A of pairs {2,3}.
"""

import sys

sys.path.insert(0, "/opt/trn_rl_repo")

import ml_dtypes
import numpy as np

import concourse.bass as bass
import concourse.bass_isa as bass_isa
import concourse.tile as tile
from concourse import bacc, mybir

T, B, D = 4096, 64, 64
N_CORES = 8
BC = B // N_CORES  # 8 batches per core
NP = BC // 2  # 4 batch pairs
NS = T // 128  # 32 subchunks of 128 tokens
NQ = 4  # quarter-groups (1024 tokens)
SQ = NS // NQ  # 8 subchunks per quarter
SCALE = float(D) ** -0.5
F32 = mybir.dt.float32
BF16 = mybir.dt.bfloat16
NPBF16 = ml_dtypes.bfloat16
AF = mybir.ActivationFunctionType
ALU = mybir.AluOpType
AX = mybir.AxisListType

_CACHE = {}


def _build_nc():
    nc = bacc.Bacc(
        "TRN2", target_bir_lowering=False, debug=False, num_devices=N_CORES
    )

    # feature-major pair-packed inputs: [pair, halfT, 128=(f_b0|f_b1), 2048]
    xp_d = nc.dram_tensor("XP", [NP, 2, 128, 2048], BF16, kind="ExternalInput").ap()
    yp_d = nc.dram_tensor("YP", [NP, 2, 128, 2048], BF16, kind="ExternalInput").ap()
    # x token-major (residual): per b: [128 tokpos, 32 sub * 64 feat]
    xt_d = nc.dram_tensor("XT", [BC, 128, NS * 64], BF16, kind="ExternalInput").ap()
    bigz2_d = nc.dram_tensor("BIGZ2", [128, 128], BF16, kind="ExternalInput").ap()
    u2_d = nc.dram_tensor("U2", [128, 1], F32, kind="ExternalInput").ap()
    rv_d = nc.dram_tensor("RV2", [128, 128], BF16, kind="ExternalInput").ap()
    rwred_d = nc.dram_tensor("RWRED", [128, 5], BF16, kind="ExternalInput").ap()
    onesrow_d = nc.dram_tensor("ONESROW", [128, 128], BF16, kind="ExternalInput").ap()
    rbv_d = nc.dram_tensor("RBV", [128, 512], BF16, kind="ExternalInput").ap()
    out_d = nc.dram_tensor("OUT", [BC, 128, NS * 64], BF16, kind="ExternalOutput").ap()

    with tile.TileContext(nc) as tc:
        with (
            tc.tile_pool(name="consts", bufs=1) as consts,
            tc.tile_pool(name="xyp", bufs=6) as xyp,
            tc.tile_pool(name="store", bufs=1) as store,
            tc.tile_pool(name="m", bufs=3) as mp,
            tc.tile_pool(name="sm", bufs=1) as smp,
            tc.tile_pool(name="ebt", bufs=3) as ebtp,
            tc.tile_pool(name="av", bufs=2) as avp,
            tc.tile_pool(name="o", bufs=3) as op_,
            tc.tile_pool(name="pz", bufs=2, space="PSUM") as pzp,
            tc.tile_pool(name="pv", bufs=2, space="PSUM") as pvp,
            tc.tile_pool(name="ps", bufs=1, space="PSUM") as psp,
        ):
            bigz2_s = consts.tile([128, 128], BF16)
            nc.sync.dma_start(bigz2_s[:], bigz2_d[:])
            u2_s = consts.tile([128, 1], F32)
            nc.sync.dma_start(u2_s[:], u2_d[:])
            rv_s = consts.tile([128, 128], BF16)
            nc.sync.dma_start(rv_s[:], rv_d[:])
            rwred_s = consts.tile([128, 5], BF16)
            nc.sync.dma_start(rwred_s[:], rwred_d[:])
            onesrow_s = consts.tile([128, 128], BF16)
            nc.sync.dma_start(onesrow_s[:], onesrow_d[:])
            rbv_s = consts.tile([128, 512], BF16)
            nc.sync.dma_start(rbv_s[:], rbv_d[:])

            # persistent stores
            xt_s = store.tile([128, BC * NS * 64], BF16)  # 32 KB/part
            # vstore paired: [128, pair, NS, (v_b0 64 | v_b1 64)]
            vstore = store.tile([128, NP * NS * 128], BF16)  # 32 KB/part
            pzacc = smp.tile([128, BC], F32)  # per-partition exp partials
            zr128 = smp.tile([128, BC], F32)
            rz128 = smp.tile([128, BC], F32)
            # scores psum banks: col = b*NS + q*SQ + s
            swp = psp.tile([128, 2 * BC * NS], F32, tag="swp")
            spsum = swp[:, 0 : BC * NS]
            wupsum = swp[:, BC * NS : 2 * BC * NS]
            wusb = smp.tile([128, BC * NS], F32)
            S = smp.tile([128, BC * NS], F32)

            def bsl(b):  # per-batch 2048-col slice (NS,64)
                return slice(b * NS * 64, (b + 1) * NS * 64)

            def pass_a(p):
                for h2 in range(2):
                    xp = xyp.tile([128, 2048], BF16, tag="xp")
                    nc.sync.dma_start(xp[:], xp_d[p, h2])
                    yp = xyp.tile([128, 2048], BF16, tag="yp")
                    nc.sync.dma_start(yp[:], yp_d[p, h2])
                    if h2 == 0:
                        nc.gpsimd.dma_start(xt_s[:, bsl(2 * p)], xt_d[2 * p])
                        nc.gpsimd.dma_start(
                            xt_s[:, bsl(2 * p + 1)], xt_d[2 * p + 1]
                        )
                    for qq in range(2):
                        q = h2 * 2 + qq
                        qsl = slice(qq * 1024, (qq + 1) * 1024)
                        zp = pzp.tile([128, 1024], F32, tag="zp")
                        for zh in range(2):
                            nc.tensor.matmul(
                                zp[:, zh * 512 : (zh + 1) * 512],
                                bigz2_s[:],
                                xp[:, qq * 1024 + zh * 512 : qq * 1024 + (zh + 1) * 512],
                                start=True, stop=True,
                            )
                        # m = (z + u2) * y  (bf16)
                        m = mp.tile([128, 1024], BF16, tag="m")
                        nc.vector.scalar_tensor_tensor(
                            m[:], zp[:], u2_s[:], yp[:, qsl],
                            op0=ALU.add, op1=ALU.mult,
                        )
                        # v-pair (token-major): bv opens the bank, v accums
                        for hh in range(2):
                            pv = pvp.tile([128, 512], F32, tag="pv")
                            nc.tensor.matmul(
                                pv[:], onesrow_s[:], rbv_s[:],
                                start=True, stop=False,
                            )
                            for si in range(4):
                                s = hh * 4 + si
                                tsl = slice(
                                    qq * 1024 + s * 128,
                                    qq * 1024 + (s + 1) * 128,
                                )
                                nc.tensor.matmul(
                                    pv[:, si * 128 : (si + 1) * 128],
                                    yp[:, tsl], rv_s[:],
                                    start=False, stop=(si == 3),
                                )
                            vq = slice(
                                (p * NS + q * SQ + hh * 4) * 128,
                                (p * NS + q * SQ + hh * 4 + 4) * 128,
                            )
                            nc.scalar.activation(
                                vstore[:, vq], pv[:], AF.Copy,
                                bias=0.0, scale=64.0,
                            )

                        # scores reduce on PE: col pair = sum(m), w.x
                        for s in range(SQ):
                            tsl = slice(
                                qq * 1024 + s * 128, qq * 1024 + (s + 1) * 128
                            )
                            msl = slice(s * 128, (s + 1) * 128)
                            col = p * 64 + (q * SQ + s) * 2
                            nc.tensor.matmul(
                                spsum[:, col : col + 2],
                                m[:, msl], rwred_s[:, 0:2],
                                start=True, stop=True,
                            )
                            nc.tensor.matmul(
                                wupsum[:, col : col + 2],
                                xp[:, tsl], rwred_s[:, 2:4],
                                start=True, stop=True,
                            )
            def softmax_pass_b(p):
                psl = slice(p * 64, (p + 1) * 64)
                nc.scalar.activation(
                    wusb[:, psl], wupsum[:, psl], AF.Copy, bias=0.0
                )
                nc.vector.tensor_tensor(
                    S[:, psl], spsum[:, psl], wusb[:, psl], op=ALU.add
                )
                ebts = []
                for bi in range(2):
                    b = 2 * p + bi
                    sin = (
                        S[:, psl]
                        .rearrange("pp (s two) -> pp two s", two=2)[
                            :, bi : bi + 1, :
                        ]
                        .squeeze(1)
                    )
                    ebt = ebtp.tile([128, NS * 64], BF16, tag="ebt")
                    ebts.append(ebt)
                    nc.scalar.activation(
                        ebt[:].rearrange("p (s f) -> p s f", f=64),
                        sin.unsqueeze(2).broadcast_to([128, NS, 64]),
                        AF.Exp,
                        bias=0.0, accum_out=pzacc[:, b : b + 1],
                    )
                b2 = slice(2 * p, 2 * p + 2)
                nc.gpsimd.partition_all_reduce(
                    zr128[:, b2], pzacc[:, b2], 128, bass_isa.ReduceOp.add
                )
                nc.vector.reciprocal(rz128[:, b2], zr128[:, b2])
                for bi in range(2):
                    b = 2 * p + bi
                    # av = (v * 1/Z_b) * E-broadcast   (one stt)
                    av = avp.tile([128, NS * 64], BF16, tag="av")
                    nc.vector.tensor_tensor(
                        av[:].rearrange("p (s f) -> p s f", f=64),
                        vstore[:, p * NS * 128 : (p + 1) * NS * 128]
                        .rearrange("p (s f) -> p s f", f=128)[
                            :, :, bi * 64 : bi * 64 + 64
                        ],
                        ebts[bi][:].rearrange("p (s f) -> p s f", f=64),
                        op=ALU.mult,
                    )
                    # o = av * 1/Z + x  (stt, bf16 out)
                    o = op_.tile([128, NS * 64], BF16, tag="o")
                    nc.vector.scalar_tensor_tensor(
                        o[:], av[:], rz128[:, b : b + 1], xt_s[:, bsl(b)],
                        op0=ALU.mult, op1=ALU.add,
                    )
                    dmae = nc.sync if b >= 6 else nc.gpsimd
                    dmae.dma_start(out_d[b], o[:])

            for p in range(NP):
                pass_a(p)
                if p >= 1:
                    softmax_pass_b(p - 1)
            softmax_pass_b(NP - 1)

    nc.compile()
    return nc


def _get_nc():
    if "nc" not in _CACHE:
        _CACHE["nc"] = _build_nc()
    return _CACHE["nc"]


def _host_constants(Wq, bq, Wk, bk, Wv, bv):
    Wq64, Wk64, Wv64 = (np.asarray(a, np.float64) for a in (Wq, Wk, Wv))
    bq64, bk64 = np.asarray(bq, np.float64), np.asarray(bk, np.float64)
    G = SCALE * (Wq64.T @ Wk64)  # z = G^T x (feature-major)
    w = SCALE * (Wq64.T @ bk64)
    u = SCALE * (Wk64.T @ bq64)

    BIGZ2 = np.zeros((128, 128), np.float64)
    BIGZ2[0:64, 0:64] = G
    BIGZ2[64:128, 64:128] = G
    U2 = np.concatenate([u, u]).reshape(128, 1).astype(np.float32)
    RV2 = np.zeros((128, 128), np.float64)  # cols 0:64 -> v_b0; 64:128 -> v_b1
    RV2[0:64, 0:64] = Wv64.T
    RV2[64:128, 64:128] = Wv64.T
    RWRED = np.zeros((128, 5), np.float64)
    RWRED[0:64, 0] = 1.0  # reduce m_b0
    RWRED[64:128, 1] = 1.0  # reduce m_b1
    RWRED[0:64, 2] = w  # + w.x_b0
    RWRED[64:128, 3] = w  # + w.x_b1
    RWRED[:, 4] = 1.0  # all-ones col for Z_b partition sum
    ONESROW = np.zeros((128, 128), np.float64)
    ONESROW[0, :] = 1.0
    RBV = np.zeros((128, 512), np.float64)
    RBV[0, :] = np.tile(np.concatenate([bv, bv]), 4)
    return (
        BIGZ2.astype(NPBF16),
        U2,
        RV2.astype(NPBF16),
        RWRED.astype(NPBF16),
        ONESROW.astype(NPBF16),
        RBV.astype(NPBF16),
    )


def _pack_inputs(eeg, fnirs):
    # XP/YP [core, pair, half, 128=(f_b0|f_b1), 2048]
    e = np.asarray(eeg, np.float32).reshape(2, 2048, N_CORES, NP, 2, D)
    f = np.asarray(fnirs, np.float32).reshape(2, 2048, N_CORES, NP, 2, D)
    # -> [core, pair, half, bi, D, tok]
    XP = np.ascontiguousarray(e.transpose(2, 3, 0, 4, 5, 1)).astype(NPBF16)
    YP = np.ascontiguousarray(f.transpose(2, 3, 0, 4, 5, 1)).astype(NPBF16)
    XP = XP.reshape(N_CORES, NP, 2, 128, 2048)
    YP = YP.reshape(N_CORES, NP, 2, 128, 2048)
    # XT [core, b, 128 tokpos, NS, 64] token-major
    et = np.asarray(eeg, np.float32).reshape(NS, 128, N_CORES, BC, D)
    XT = np.ascontiguousarray(et.transpose(2, 3, 1, 0, 4)).astype(NPBF16)
    return XP, YP, XT.reshape(N_CORES, BC, 128, NS * 64)


def _unpack_output(outs):
    o = np.stack(outs).astype(np.float32).reshape(N_CORES, BC, 128, NS, D)
    o = o.transpose(3, 2, 0, 1, 4)  # [sub, tokpos, core, b, feat]
    return np.ascontiguousarray(o.reshape(T, B, D))


def _prepare(eeg, fnirs, Wq, bq, Wk, bk, Wv, bv):
    BIGZ2, U2, RV2, RWRED, ONESROW, RBV = _host_constants(Wq, bq, Wk, bk, Wv, bv)
    XP, YP, XT = _pack_inputs(eeg, fnirs)
    return [
        {
            "XP": XP[c],
            "YP": YP[c],
            "XT": XT[c],
            "BIGZ2": BIGZ2,
            "U2": U2,
            "RV2": RV2,
            "RWRED": RWRED,
            "ONESROW": ONESROW,
            "RBV": RBV,
        }
        for c in range(N_CORES)
    ]


def _run(eeg, fnirs, Wq, bq, Wk, bk, Wv, bv, **spmd_kwargs):
    from concourse.bass_utils import run_bass_kernel_spmd

    nc = _get_nc()
    in_maps = _prepare(eeg, fnirs, Wq, bq, Wk, bk, Wv, bv)
    res = run_bass_kernel_spmd(nc, in_maps, list(range(N_CORES)), **spmd_kwargs)
    return _unpack_output([res.results[c]["OUT"] for c in range(N_CORES)]), res


def kernel(eeg, fnirs, Wq, bq, Wk, bk, Wv, bv):
    return _run(eeg, fnirs, Wq, bq, Wk, bk, Wv, bv)[0]



# revision 30
# speedup vs baseline: 1.1032x; 1.1032x over previous
"""CrossAttentionFusion Bass kernel v2 — batch-pair / token-major hybrid.

Reference (T=4096, B=64, D=64):
    q = eeg @ Wq.T + bq ; k = fnirs @ Wk.T + bk ; v = fnirs @ Wv.T + bv
    s = sum(q*k, -1) * D**-0.5 ; a = softmax(s, axis=0) ; out = eeg + a*v

Algebra: s = x^T G y + w.x + u.y, G = SCALE*Wq^T Wk, w = SCALE*Wq^T bk,
u = SCALE*Wk^T bq.

Core ideas (per core: 8 batches, all 4096 tokens):
  - Batches processed in PAIRS (b0,b1) stacked on partitions: host packs
    XP=[x0;x1], YP=[y0;y1] (feature-major bf16 [128, tok]).
  - PE: z-pair = blockdiag(G,G)^T @ XP   [128=z0|z1, 1024] one N=1024 mm.
  - DVE: m = (z + u2) * YP  — ONE scalar_tensor_tensor per 1024 tokens
    (PSUM x SBUF, u per-partition in this layout), bf16 out.
  - PE reduces scores: per 128-token slice, lhsT=m-tile with rhs
    [1_64;0] / [0;1_64] (N=1) accumulated with w.x via rhs [w;0] / [0;w]
    second matmul into a per-token scores PSUM bank [128tok, 256 cols].
  - v-pair = YP-slice^T @ [[Wv^T],[0]] / [[0],[Wv^T]] (token-major v),
    + bv via ones-row matmul; scalar ACT evacuates to bf16 vstore.
  - Softmax over T per batch: no max-sub (|s|~4); exp ACT reads scores
    PSUM with accum_out; GpSimd C-reduce -> Z_b; reciprocal;
    gpsimd.partition_broadcast for 1/Z.
  - Pass B token-major: ABT_b = E_b*(1/Z_b) broadcast across 64 feats
    (ONE stride-0 scalar ACT per batch), av = ABT*v (DVE), o = av + x
    (XT host-packed token-major bf16; DVE/GpSimd alternating), DMA out
    on the scalar HWDGE ring (inputs ride the sync ring).
  - Two batch-half pipeline: softmax+passB of pairs {0,1} overlaps
    pass A of pairs {2,3}.
"""

import sys

sys.path.insert(0, "/opt/trn_rl_repo")

import ml_dtypes
import numpy as np

import concourse.bass as bass
import concourse.bass_isa as bass_isa
import concourse.tile as tile
from concourse import bacc, mybir

T, B, D = 4096, 64, 64
N_CORES = 8
BC = B // N_CORES  # 8 batches per core
NP = BC // 2  # 4 batch pairs
NS = T // 128  # 32 subchunks of 128 tokens
NQ = 4  # quarter-groups (1024 tokens)
SQ = NS // NQ  # 8 subchunks per quarter
SCALE = float(D) ** -0.5
F32 = mybir.dt.float32
BF16 = mybir.dt.bfloat16
NPBF16 = ml_dtypes.bfloat16
AF = mybir.ActivationFunctionType
ALU = mybir.AluOpType
AX = mybir.AxisListType

_CACHE = {}


def _build_nc():
    nc = bacc.Bacc(
        "TRN2", target_bir_lowering=False, debug=False, num_devices=N_CORES
    )

    # feature-major pair-packed inputs: [pair, halfT, 128=(f_b0|f_b1), 2048]
    xp_d = nc.dram_tensor("XP", [NP, 2, 128, 2048], BF16, kind="ExternalInput").ap()
    yp_d = nc.dram_tensor("YP", [NP, 2, 128, 2048], BF16, kind="ExternalInput").ap()
    # x token-major (residual): per b: [128 tokpos, 32 sub * 64 feat]
    xt_d = nc.dram_tensor("XT", [BC, 128, NS * 64], BF16, kind="ExternalInput").ap()
    bigz2_d = nc.dram_tensor("BIGZ2", [128, 128], BF16, kind="ExternalInput").ap()
    u2_d = nc.dram_tensor("U2", [128, 1], F32, kind="ExternalInput").ap()
    rv_d = nc.dram_tensor("RV2", [128, 128], BF16, kind="ExternalInput").ap()
    rwred_d = nc.dram_tensor("RWRED", [128, 5], BF16, kind="ExternalInput").ap()
    onesrow_d = nc.dram_tensor("ONESROW", [128, 128], BF16, kind="ExternalInput").ap()
    rbv_d = nc.dram_tensor("RBV", [128, 512], BF16, kind="ExternalInput").ap()
    out_d = nc.dram_tensor("OUT", [BC, 128, NS * 64], BF16, kind="ExternalOutput").ap()

    with tile.TileContext(nc) as tc:
        with (
            tc.tile_pool(name="consts", bufs=1) as consts,
            tc.tile_pool(name="xyp", bufs=6) as xyp,
            tc.tile_pool(name="store", bufs=1) as store,
            tc.tile_pool(name="m", bufs=3) as mp,
            tc.tile_pool(name="sm", bufs=1) as smp,
            tc.tile_pool(name="ebt", bufs=3) as ebtp,
            tc.tile_pool(name="av", bufs=2) as avp,
            tc.tile_pool(name="o", bufs=3) as op_,
            tc.tile_pool(name="pz", bufs=2, space="PSUM") as pzp,
            tc.tile_pool(name="pv", bufs=2, space="PSUM") as pvp,
            tc.tile_pool(name="ps", bufs=1, space="PSUM") as psp,
        ):
            bigz2_s = consts.tile([128, 128], BF16)
            nc.sync.dma_start(bigz2_s[:], bigz2_d[:])
            u2_s = consts.tile([128, 1], F32)
            nc.sync.dma_start(u2_s[:], u2_d[:])
            rv_s = consts.tile([128, 128], BF16)
            nc.sync.dma_start(rv_s[:], rv_d[:])
            rwred_s = consts.tile([128, 5], BF16)
            nc.sync.dma_start(rwred_s[:], rwred_d[:])
            onesrow_s = consts.tile([128, 128], BF16)
            nc.sync.dma_start(onesrow_s[:], onesrow_d[:])
            rbv_s = consts.tile([128, 512], BF16)
            nc.sync.dma_start(rbv_s[:], rbv_d[:])

            # persistent stores
            xt_s = store.tile([128, BC * NS * 64], BF16)  # 32 KB/part
            # vstore paired: [128, pair, NS, (v_b0 64 | v_b1 64)]
            vstore = store.tile([128, NP * NS * 128], BF16)  # 32 KB/part
            pzacc = smp.tile([128, BC], F32)  # per-partition exp partials
            zr128 = smp.tile([128, BC], F32)
            rz128 = smp.tile([128, BC], F32)
            # scores psum bank: col = b*NS + q*SQ + s (w.x accumulated in)
            swp = psp.tile([128, BC * NS], F32, tag="swp")
            spsum = swp[:, 0 : BC * NS]

            def bsl(b):  # per-batch 2048-col slice (NS,64)
                return slice(b * NS * 64, (b + 1) * NS * 64)

            def pass_a(p):
                for h2 in range(2):
                    xp = xyp.tile([128, 2048], BF16, tag="xp")
                    nc.sync.dma_start(xp[:], xp_d[p, h2])
                    yp = xyp.tile([128, 2048], BF16, tag="yp")
                    nc.sync.dma_start(yp[:], yp_d[p, h2])
                    if h2 == 0:
                        nc.scalar.dma_start(xt_s[:, bsl(2 * p)], xt_d[2 * p])
                        nc.scalar.dma_start(
                            xt_s[:, bsl(2 * p + 1)], xt_d[2 * p + 1]
                        )
                    for qq in range(2):
                        q = h2 * 2 + qq
                        qsl = slice(qq * 1024, (qq + 1) * 1024)
                        zp = pzp.tile([128, 1024], F32, tag="zp")
                        for zh in range(2):
                            nc.tensor.matmul(
                                zp[:, zh * 512 : (zh + 1) * 512],
                                bigz2_s[:],
                                xp[:, qq * 1024 + zh * 512 : qq * 1024 + (zh + 1) * 512],
                                start=True, stop=True,
                            )
                        # m = (z + u2) * y  (bf16)
                        m = mp.tile([128, 1024], BF16, tag="m")
                        nc.vector.scalar_tensor_tensor(
                            m[:], zp[:], u2_s[:], yp[:, qsl],
                            op0=ALU.add, op1=ALU.mult,
                        )
                        # v-pair (token-major): bv opens the bank, v accums
                        for hh in range(2):
                            pv = pvp.tile([128, 512], F32, tag="pv")
                            nc.tensor.matmul(
                                pv[:], onesrow_s[:], rbv_s[:],
                                start=True, stop=False,
                            )
                            for si in range(4):
                                s = hh * 4 + si
                                tsl = slice(
                                    qq * 1024 + s * 128,
                                    qq * 1024 + (s + 1) * 128,
                                )
                                nc.tensor.matmul(
                                    pv[:, si * 128 : (si + 1) * 128],
                                    yp[:, tsl], rv_s[:],
                                    start=False, stop=(si == 3),
                                )
                            vq = slice(
                                (p * NS + q * SQ + hh * 4) * 128,
                                (p * NS + q * SQ + hh * 4 + 4) * 128,
                            )
                            nc.scalar.activation(
                                vstore[:, vq], pv[:], AF.Copy,
                                bias=0.0, scale=64.0,
                            )

                        # scores reduce on PE: col pair = sum(m), w.x
                        for s in range(SQ):
                            tsl = slice(
                                qq * 1024 + s * 128, qq * 1024 + (s + 1) * 128
                            )
                            msl = slice(s * 128, (s + 1) * 128)
                            col = p * 64 + (q * SQ + s) * 2
                            nc.tensor.matmul(
                                spsum[:, col : col + 2],
                                m[:, msl], rwred_s[:, 0:2],
                                start=True, stop=False,
                            )
                            nc.tensor.matmul(
                                spsum[:, col : col + 2],
                                xp[:, tsl], rwred_s[:, 2:4],
                                start=False, stop=True,
                            )
            def softmax_pass_b(p):
                psl = slice(p * 64, (p + 1) * 64)
                ebts = []
                for bi in range(2):
                    b = 2 * p + bi
                    sin = (
                        spsum[:, psl]
                        .rearrange("pp (s two) -> pp two s", two=2)[
                            :, bi : bi + 1, :
                        ]
                        .squeeze(1)
                    )
                    ebt = ebtp.tile([128, NS * 64], BF16, tag="ebt")
                    ebts.append(ebt)
                    nc.scalar.activation(
                        ebt[:].rearrange("p (s f) -> p s f", f=64),
                        sin.unsqueeze(2).broadcast_to([128, NS, 64]),
                        AF.Exp,
                        bias=0.0, accum_out=pzacc[:, b : b + 1],
                    )
                b2 = slice(2 * p, 2 * p + 2)
                nc.gpsimd.partition_all_reduce(
                    zr128[:, b2], pzacc[:, b2], 128, bass_isa.ReduceOp.add
                )
                nc.vector.reciprocal(rz128[:, b2], zr128[:, b2])
                for bi in range(2):
                    b = 2 * p + bi
                    # av = (v * 1/Z_b) * E-broadcast   (one stt)
                    av = avp.tile([128, NS * 64], BF16, tag="av")
                    nc.vector.tensor_tensor(
                        av[:].rearrange("p (s f) -> p s f", f=64),
                        vstore[:, p * NS * 128 : (p + 1) * NS * 128]
                        .rearrange("p (s f) -> p s f", f=128)[
                            :, :, bi * 64 : bi * 64 + 64
                        ],
                        ebts[bi][:].rearrange("p (s f) -> p s f", f=64),
                        op=ALU.mult,
                    )
                    # o = av * 1/Z + x  (stt, bf16 out)
                    o = op_.tile([128, NS * 64], BF16, tag="o")
                    nc.vector.scalar_tensor_tensor(
                        o[:], av[:], rz128[:, b : b + 1], xt_s[:, bsl(b)],
                        op0=ALU.mult, op1=ALU.add,
                    )
                    dmae = nc.sync
                    dmae.dma_start(out_d[b], o[:])

            for p in range(NP):
                pass_a(p)
                if p >= 1:
                    softmax_pass_b(p - 1)
            softmax_pass_b(NP - 1)

    nc.compile()
    return nc


def _get_nc():
    if "nc" not in _CACHE:
        _CACHE["nc"] = _build_nc()
    return _CACHE["nc"]


def _host_constants(Wq, bq, Wk, bk, Wv, bv):
    Wq64, Wk64, Wv64 = (np.asarray(a, np.float64) for a in (Wq, Wk, Wv))
    bq64, bk64 = np.asarray(bq, np.float64), np.asarray(bk, np.float64)
    G = SCALE * (Wq64.T @ Wk64)  # z = G^T x (feature-major)
    w = SCALE * (Wq64.T @ bk64)
    u = SCALE * (Wk64.T @ bq64)

    BIGZ2 = np.zeros((128, 128), np.float64)
    BIGZ2[0:64, 0:64] = G
    BIGZ2[64:128, 64:128] = G
    U2 = np.concatenate([u, u]).reshape(128, 1).astype(np.float32)
    RV2 = np.zeros((128, 128), np.float64)  # cols 0:64 -> v_b0; 64:128 -> v_b1
    RV2[0:64, 0:64] = Wv64.T
    RV2[64:128, 64:128] = Wv64.T
    RWRED = np.zeros((128, 5), np.float64)
    RWRED[0:64, 0] = 1.0  # reduce m_b0
    RWRED[64:128, 1] = 1.0  # reduce m_b1
    RWRED[0:64, 2] = w  # + w.x_b0
    RWRED[64:128, 3] = w  # + w.x_b1
    RWRED[:, 4] = 1.0  # all-ones col for Z_b partition sum
    ONESROW = np.zeros((128, 128), np.float64)
    ONESROW[0, :] = 1.0
    RBV = np.zeros((128, 512), np.float64)
    RBV[0, :] = np.tile(np.concatenate([bv, bv]), 4)
    return (
        BIGZ2.astype(NPBF16),
        U2,
        RV2.astype(NPBF16),
        RWRED.astype(NPBF16),
        ONESROW.astype(NPBF16),
        RBV.astype(NPBF16),
    )


def _pack_inputs(eeg, fnirs):
    # XP/YP [core, pair, half, 128=(f_b0|f_b1), 2048]
    e = np.asarray(eeg, np.float32).reshape(2, 2048, N_CORES, NP, 2, D)
    f = np.asarray(fnirs, np.float32).reshape(2, 2048, N_CORES, NP, 2, D)
    # -> [core, pair, half, bi, D, tok]
    XP = np.ascontiguousarray(e.transpose(2, 3, 0, 4, 5, 1)).astype(NPBF16)
    YP = np.ascontiguousarray(f.transpose(2, 3, 0, 4, 5, 1)).astype(NPBF16)
    XP = XP.reshape(N_CORES, NP, 2, 128, 2048)
    YP = YP.reshape(N_CORES, NP, 2, 128, 2048)
    # XT [core, b, 128 tokpos, NS, 64] token-major
    et = np.asarray(eeg, np.float32).reshape(NS, 128, N_CORES, BC, D)
    XT = np.ascontiguousarray(et.transpose(2, 3, 1, 0, 4)).astype(NPBF16)
    return XP, YP, XT.reshape(N_CORES, BC, 128, NS * 64)


def _unpack_output(outs):
    o = np.stack(outs).astype(np.float32).reshape(N_CORES, BC, 128, NS, D)
    o = o.transpose(3, 2, 0, 1, 4)  # [sub, tokpos, core, b, feat]
    return np.ascontiguousarray(o.reshape(T, B, D))


def _prepare(eeg, fnirs, Wq, bq, Wk, bk, Wv, bv):
    BIGZ2, U2, RV2, RWRED, ONESROW, RBV = _host_constants(Wq, bq, Wk, bk, Wv, bv)
    XP, YP, XT = _pack_inputs(eeg, fnirs)
    return [
        {
            "XP": XP[c],
            "YP": YP[c],
            "XT": XT[c],
            "BIGZ2": BIGZ2,
            "U2": U2,
            "RV2": RV2,
            "RWRED": RWRED,
            "ONESROW": ONESROW,
            "RBV": RBV,
        }
        for c in range(N_CORES)
    ]


def _run(eeg, fnirs, Wq, bq, Wk, bk, Wv, bv, **spmd_kwargs):
    from concourse.bass_utils import run_bass_kernel_spmd

    nc = _get_nc()
    in_maps = _prepare(eeg, fnirs, Wq, bq, Wk, bk, Wv, bv)
    res = run_bass_kernel_spmd(nc, in_maps, list(range(N_CORES)), **spmd_kwargs)
    return _unpack_output([res.results[c]["OUT"] for c in range(N_CORES)]), res


def kernel(eeg, fnirs, Wq, bq, Wk, bk, Wv, bv):
    return _run(eeg, fnirs, Wq, bq, Wk, bk, Wv, bv)[0]

